# revision 1
# baseline (speedup 1.0000x reference)
"""Routed 3-expert MLP (512 -> 2048 -> 1) for Trainium2, 8 NeuronCores.

Strategy:
  - Host routes: nodes are grouped by expert (atomic_num 6 -> c, 1 -> h,
    else -> o) so the device only computes the selected expert per node
    (1/3 the FLOPs of the dense reference).
  - Each expert's nodes are split evenly across the 8 cores; experts are
    replicated, so no cross-device traffic.
  - Host pre-transposes x into [128, 4, N] bf16 so matmul stationary
    tiles stream straight from DRAM with no on-chip transposes.
  - sign(W2) is folded into a hidden-unit permutation (positive-sign
    units first) and |W2| is folded into W1/b1.  With
    relu(z + b) = max(z, -b) + b, the whole second layer collapses to
        out = sum_P max(z,-b1') - sum_N max(z,-b1') + const_e
    where const_e = sum_h sign(w2_h) b1'_h + b2 is added on the host.
  - Per 128-node tile the device does 16 matmuls (PE) and two fused
    scalar_tensor_tensor ops (DVE) with a per-partition accumulator;
    a [P, 1] subtract writes the final per-node value.

Set env KERNEL_TRACE=1 to capture an NTFF profile; the BassKernelResults
is stored in kernel._LAST_RESULT.
"""

import os
import sys
import types
import contextlib
import ctypes

import numpy as np
import ml_dtypes

P = 128          # partitions
FS = 512         # matmul free-dim slice (one PSUM bank of fp32)
NCORES = 8

_LAST_RESULT = None


def _install_ntff_hook():
    """Provide antenv.axon_hooks.get_axon_ntff_profile_hook if the agent
    image lacks it (mirrors trn_agent_boot's ctypes hook)."""
    try:
        from antenv.axon_hooks import get_axon_ntff_profile_hook  # noqa: F401
        return
    except ImportError:
        pass

    so_path = "/opt/axon/libaxon_pjrt.so"
    if not os.path.exists(so_path):
        return
    try:
        lib = ctypes.CDLL(so_path)
        if not hasattr(lib, "axon_start_nrt_profile"):
            return
        lib.axon_start_nrt_profile.argtypes = [
            ctypes.POINTER(ctypes.c_int64),
            ctypes.c_size_t,
        ]
        lib.axon_start_nrt_profile.restype = ctypes.c_int64
        lib.axon_stop_nrt_profile.argtypes = [ctypes.c_char_p]
        lib.axon_stop_nrt_profile.restype = ctypes.c_int64
    except OSError:
        return

    @contextlib.contextmanager
    def _hook(output_dir, device_ids):
        import jax

        jax.devices()
        if device_ids:
            ids = (ctypes.c_int64 * len(device_ids))(*device_ids)
            rc = lib.axon_start_nrt_profile(ids, len(device_ids))
        else:
            rc = lib.axon_start_nrt_profile(None, 0)
        if rc != 0:
            raise RuntimeError(f"axon_start_nrt_profile rc={rc}")
        try:
            yield
        finally:
            n = lib.axon_stop_nrt_profile(str(output_dir).encode())
            print(f"ntff profile: {n} file(s) -> {output_dir}", file=sys.stderr)

    mod = types.ModuleType("antenv.axon_hooks")
    mod.get_axon_ntff_profile_hook = lambda: _hook
    mod.set_axon_ntff_profile_hook = lambda h: None
    sys.modules["antenv.axon_hooks"] = mod


def _prep_expert(W1, b1, W2, b2):
    """Fold |w2| into W1/b1, permute hidden units so sign(w2)>=0 comes
    first.  Returns (w1_dev bf16 [P, KC, DH], negb1_dev f32 [P, DH],
    a_split, const)."""
    DH, DIN = W1.shape
    KC = DIN // P
    w2 = np.asarray(W2, np.float64).reshape(-1)
    b1 = np.asarray(b1, np.float64)
    pos = w2 >= 0.0
    perm = np.concatenate([np.nonzero(pos)[0], np.nonzero(~pos)[0]])
    a_split = int(pos.sum())
    absw = np.abs(w2)[perm]
    W1p = (np.abs(w2)[:, None] * np.asarray(W1, np.float64))[perm]  # [DH, DIN]
    b1p = (np.abs(w2) * b1)[perm]                                   # [DH]
    const = float(b1p[:a_split].sum() - b1p[a_split:].sum() + float(np.asarray(b2).reshape(-1)[0]))
    w1_dev = np.ascontiguousarray(
        W1p.T.reshape(KC, P, DH).transpose(1, 0, 2)
    ).astype(ml_dtypes.bfloat16)
    negb1_dev = np.ascontiguousarray(
        np.broadcast_to((-b1p).astype(np.float32), (P, DH))
    )
    return w1_dev, negb1_dev, a_split, const


def kernel(x, atomic_nums, cW1, cb1, cW2, cb2, hW1, hb1, hW2, hb2,
           oW1, ob1, oW2, ob2):
    global _LAST_RESULT

    import concourse.bass as bass  # noqa: F401
    import concourse.mybir as mybir
    from concourse import bacc
    from concourse.tile import TileContext
    from concourse.bass_utils import run_bass_kernel_spmd

    trace = os.environ.get("KERNEL_TRACE", "") not in ("", "0")
    if trace:
        _install_ntff_hook()

    x = np.asarray(x, np.float32)
    an = np.asarray(atomic_nums).reshape(-1)
    N, DIN = x.shape
    KC = DIN // P
    experts = [(cW1, cb1, cW2, cb2), (hW1, hb1, hW2, hb2), (oW1, ob1, oW2, ob2)]
    DH = np.asarray(cW1).shape[0]
    NF = DH // FS

    # ---- routing on host -------------------------------------------------
    eid = np.full(N, 2, np.int32)
    eid[an == 6] = 0
    eid[an == 1] = 1
    splits = []  # [expert][core] -> global node indices
    for e in range(3):
        ids = np.nonzero(eid == e)[0]
        splits.append(np.array_split(ids, NCORES))
    Pe = []  # padded per-core node count per expert (same on all cores)
    for e in range(3):
        mx = max(len(s) for s in splits[e])
        Pe.append(((mx + P - 1) // P) * P)
    NPAD = sum(Pe)
    T = NPAD // P

    tile_expert = []
    for e in range(3):
        tile_expert += [e] * (Pe[e] // P)

    # ---- per-expert folded weights --------------------------------------
    w1_dev, negb1_dev, a_split, const_e = [], [], [], []
    for e in range(3):
        w, nb, a, c = _prep_expert(*experts[e])
        w1_dev.append(w)
        negb1_dev.append(nb)
        a_split.append(a)
        const_e.append(c)

    # ---- per-core packed inputs -----------------------------------------
    bf16 = ml_dtypes.bfloat16
    in_maps = []
    core_real = []  # (positions within NPAD, global ids, expert) per core
    for c in range(NCORES):
        xcols = np.zeros((NPAD, DIN), np.float32)
        regions = []
        pos = 0
        for e in range(3):
            ids = splits[e][c]
            xcols[pos:pos + len(ids)] = x[ids]
            regions.append((pos, ids, e))
            pos += Pe[e]
        xt = np.ascontiguousarray(
            xcols.T.reshape(KC, P, NPAD).transpose(1, 0, 2)
        ).astype(bf16)
        m = {"xT": xt}
        for e in range(3):
            m[f"w1_{e}"] = w1_dev[e]
            m[f"nb1_{e}"] = negb1_dev[e]
        in_maps.append(m)
        core_real.append(regions)

    # ---- device program (same for every core) ---------------------------
    nc = bacc.Bacc()
    xT_h = nc.declare_dram_parameter("xT", [P, KC, NPAD], mybir.dt.bfloat16, isOutput=False)
    w1_h = [nc.declare_dram_parameter(f"w1_{e}", [P, KC, DH], mybir.dt.bfloat16, isOutput=False)
            for e in range(3)]
    nb1_h = [nc.declare_dram_parameter(f"nb1_{e}", [P, DH], mybir.dt.float32, isOutput=False)
             for e in range(3)]
    out_h = nc.declare_dram_parameter("out", [P, T], mybir.dt.float32, isOutput=True)

    BS = 512  # node block per x DMA
    Alu = mybir.AluOpType

    with TileContext(nc) as tc:
        with (
            tc.tile_pool(name="weights", bufs=1) as wpool,
            tc.tile_pool(name="xin", bufs=3) as xpool,
            tc.tile_pool(name="ps", bufs=2, space="PSUM") as ppool,
            tc.tile_pool(name="scr", bufs=1) as spool,
            tc.tile_pool(name="acc", bufs=4) as apool,
        ):
            w1sb, nb1sb = [], []
            for e in range(3):
                wt = wpool.tile([P, KC, DH], mybir.dt.bfloat16, tag=f"w1_{e}")
                nc.sync.dma_start(wt[:], w1_h[e][:])
                w1sb.append(wt)
                nt = wpool.tile([P, DH], mybir.dt.float32, tag=f"nb1_{e}")
                nc.sync.dma_start(nt[:], nb1_h[e][:])
                nb1sb.append(nt)
            outsb = wpool.tile([P, T], mybir.dt.float32, tag="outsb")
            scr = spool.tile([P, DH], mybir.dt.bfloat16, tag="scr")

            t = 0
            for nb in range(0, NPAD, BS):
                bs = min(BS, NPAD - nb)
                xt_sb = xpool.tile([P, KC, BS], mybir.dt.bfloat16, tag="xsb")
                nc.sync.dma_start(xt_sb[:, :, :bs], xT_h[:, :, nb:nb + bs])
                for ti in range(bs // P):
                    e = tile_expert[t]
                    ps = ppool.tile([P, DH], mybir.dt.float32, tag="ps")
                    for f in range(NF):
                        for k in range(KC):
                            nc.tensor.matmul(
                                ps[:, f * FS:(f + 1) * FS],
                                lhsT=xt_sb[:, k, ti * P:(ti + 1) * P],
                                rhs=w1sb[e][:, k, f * FS:(f + 1) * FS],
                                start=(k == 0),
                                stop=(k == KC - 1),
                            )
                    a = a_split[e]
                    accP = apool.tile([P, 1], mybir.dt.float32, tag="accP")
                    accN = apool.tile([P, 1], mybir.dt.float32, tag="accN")
                    if a > 0:
                        nc.vector.scalar_tensor_tensor(
                            out=scr[:, :a], in0=ps[:, :a], scalar=0.0,
                            in1=nb1sb[e][:, :a],
                            op0=Alu.bypass, op1=Alu.max, accum_out=accP[:],
                        )
                    if a < DH:
                        nc.vector.scalar_tensor_tensor(
                            out=scr[:, a:], in0=ps[:, a:], scalar=0.0,
                            in1=nb1sb[e][:, a:],
                            op0=Alu.bypass, op1=Alu.max, accum_out=accN[:],
                        )
                    if 0 < a < DH:
                        nc.vector.tensor_tensor(
                            out=outsb[:, t:t + 1], in0=accP[:], in1=accN[:],
                            op=Alu.subtract,
                        )
                    elif a == DH:
                        nc.vector.tensor_copy(outsb[:, t:t + 1], accP[:])
                    else:
                        nc.vector.tensor_scalar_mul(outsb[:, t:t + 1], accN[:], -1.0)
                    t += 1
            nc.sync.dma_start(out_h[:], outsb[:])
    nc.finalize()

    res = run_bass_kernel_spmd(nc, in_maps, list(range(NCORES)), trace=trace)
    _LAST_RESULT = res

    # ---- unshard ---------------------------------------------------------
    full = np.empty((N, 1), np.float32)
    for c in range(NCORES):
        arr = np.asarray(res.results[c]["out"])      # [P, T]
        vals = np.ascontiguousarray(arr.T).reshape(-1)  # node-order within core
        for pos, ids, e in core_real[c]:
            if len(ids):
                full[ids, 0] = vals[pos:pos + len(ids)] + np.float32(const_e[e])
    return full
